# revision 77
# baseline (speedup 1.0000x reference)
"""MoE (dense routing) Trainium2 kernel.

Math: out = softmax(x@Wg+bg) -weighted sum over experts of
      (gelu(x@W1[e]+b1[e]) @ W2[e] + b2[e]).

Strategy (data-parallel over 8 cores, 2048 tokens each):
  - Host pre-transposes x (xT [D, tokens]) and packs W1 as [D, E*H];
    all matmul operands are converted to bf16 on host (tolerance is
    2e-2 rel; bf16 lands ~4e-3). bf16 halves every DMA (the weight
    prologue is HBM-bandwidth-bound at ~350GB/s) and enables the PE's
    Fast Weight Load path (fp32 LDWEIGHTS at ~191ns was pacing L1's
    107ns matmuls).
  - Layer 1 runs "transposed": hT[ej, t] = sum_d W1p[d, ej] * xT[d, t]
    via matmuls with W1p chunks stationary and xT chunks moving ->
    hidden lands with ej on partitions, tokens on free dim.
  - b1 is applied as the ACT bias during the gelu (per-partition bias).
  - Gate: logitsT[e, t] accumulated the same way; exp fused with +bg on
    ACT; weights kept UNNORMALIZED (exp). The softmax denominator is
    applied at the very end as a per-token scale on the output copy
    (on DVE, so the ACT engine never loads the Copy table and PSUM
    output banks recycle without waiting on the gelu stream).
  - Scaled hidden shT[ej, t] = gelu_out * exp[e(ej), t] (DVE mul with a
    DMA partition-broadcast of the exp row).
  - Layer 2: out[t, o] = sum_ej shT[ej, t(chunk stationary)] @ W2p[ej, o]
    accumulated in PSUM over all ej chunks, seeded with expT @ b2
    (start=True) which realizes the sum_e w_e*b2[e] term. The two
    K=8 bias seeds of a 128-token slice run CONCURRENTLY in separate
    PE row groups (tile_position row tiling) -- exp and b2 are
    replicated at partition offset 32 so the second matmul's operands
    stream through array rows 32-63.
  - PE program order per iteration: l1(k) matmuls, gate logits(k+1),
    l2(k-1), softmax-denominator(k+1). The denominator matmul's
    stationary operand is ACT's exp output; putting it after l2 gives
    the ACT queue (16 gelus + an Exp<->Gelu table reload) a full l2
    stage of slack.
  - DMA prologue: first bytes on each queue are exactly what the PE
    consumes first (xts0+Wgs on SP; W1 chunks round-robined in
    consumption order across Pool/SP/ACT). Aggregate HBM is ~350GB/s,
    so the 4MB W1 takes ~12us; the PE starts on chunk 0 at ~9us and
    never outruns the stream.
No transposes on device at all.
"""

import numpy as np
from contextlib import ExitStack

import ml_dtypes
import orjson

import concourse.bass as bass
import concourse.bass2jax as bass2jax
import concourse.bass_utils as bass_utils
import concourse.tile as tile
from concourse import mybir
from concourse.bass_utils import run_bass_kernel_spmd

# The walrus build in this container rejects any instruction carrying more
# than one sync wait ("Too many sync wait commands", CoreV3GenImpl
# setupSyncWait), but the tile scheduler freely attaches several. Split the
# extras onto standalone single-wait EventSemaphore carriers placed just
# before the instruction (same engine, so program order is preserved).
_orig_compile_bir_kernel = bass_utils.compile_bir_kernel


def _split_multiwait_bir(bir_json):
    bir = orjson.loads(bir_json)
    changed = False
    for fn in bir.get("functions", []):
        for blk in fn.get("blocks", []):
            ins_list = blk.get("instructions")
            if not ins_list:
                continue
            out = []
            for inst in ins_list:
                si = inst.get("sync_info")
                if si:
                    waits = si.get("on_wait") or []
                    if len(waits) > 1:
                        changed = True
                        for k, w in enumerate(waits[:-1]):
                            carrier = {
                                "engine": inst["engine"],
                                "ins": [],
                                "outs": [],
                                "name": f"{inst['name']}_xw{k}",
                                "opcode": "EventSemaphore",
                                "sync_info": {"on_update": [], "on_wait": [w]},
                            }
                            if "debug" in inst:
                                carrier["debug"] = inst["debug"]
                            out.append(carrier)
                        si["on_wait"] = [waits[-1]]
                out.append(inst)
            blk["instructions"] = out
    return orjson.dumps(bir) if changed else bir_json


def _compile_bir_kernel_split(bir_json, tmpdir, neff_name="file.neff"):
    return _orig_compile_bir_kernel(_split_multiwait_bir(bir_json), tmpdir, neff_name)


bass_utils.compile_bir_kernel = _compile_bir_kernel_split
bass2jax.compile_bir_kernel = _compile_bir_kernel_split

N, D, H, O, E = 16384, 1024, 256, 1024, 8
NCORES = 8
NTOK = N // NCORES  # tokens per core
P = 128
T = 512  # token block size (moving free dim)
TS = T // P  # 128-token sub-blocks per block
NB = NTOK // T  # token blocks per core
DC = D // P  # d chunks (contraction, layer 1)
EJ = E * H  # packed hidden width
NEJ = EJ // P  # ej chunks (contraction, layer 2)
JC_PER_E = H // P  # ej chunks per expert
OH = O // 2  # layer-2 output half width (one PSUM bank)

FP = mybir.dt.float32
FPR = mybir.dt.float32r
BF = mybir.dt.bfloat16
AF = mybir.ActivationFunctionType
NPBF = ml_dtypes.bfloat16


NCH = 8  # W1 column chunks (two ejc each)
CW = EJ // NCH  # 256 cols per chunk


def _build_nc():
    nc = bass.Bass(enable_partition_id=False)
    # xT and W1 are packed PARTITION-OUTER on host so every DMA slice is a
    # long contiguous run per partition (512B-row patterns measured ~97GB/s
    # per queue; 2-4KB rows ~210GB/s)
    xT = nc.dram_tensor("xT", [P, NB, DC, T], BF, kind="ExternalInput")
    W1p = nc.dram_tensor("W1p", [P, NCH, DC, CW], BF, kind="ExternalInput")
    Wg = nc.dram_tensor("Wg", [P, DC, E], BF, kind="ExternalInput")
    W2p = nc.dram_tensor("W2p", [EJ, O], BF, kind="ExternalInput")
    # b1 (transposed) and bg packed into one fp32 blob: one DMA dispatch
    # (~0.8us of queue time each) instead of two
    cpk = nc.dram_tensor("cpk", [P, NEJ + 1], FP, kind="ExternalInput")
    cmat = nc.dram_tensor("cmat", [P, E], FPR, kind="ExternalInput")
    b2 = nc.dram_tensor("b2", [E, O], BF, kind="ExternalInput")
    out = nc.dram_tensor("out", [NTOK, O], FP, kind="ExternalOutput")

    with tile.TileContext(nc) as tc, ExitStack() as ctx:
        const = ctx.enter_context(tc.tile_pool(name="const", bufs=1))
        dpool = ctx.enter_context(tc.tile_pool(name="dram", bufs=2, space="DRAM"))
        xpool = ctx.enter_context(tc.tile_pool(name="xts", bufs=4))
        gpool = ctx.enter_context(tc.tile_pool(name="gelu", bufs=6))
        shpool = ctx.enter_context(tc.tile_pool(name="sh", bufs=2))
        bcpool = ctx.enter_context(tc.tile_pool(name="bc", bufs=2))
        epool = ctx.enter_context(tc.tile_pool(name="expp", bufs=3))
        eqpool = ctx.enter_context(tc.tile_pool(name="eq", bufs=2))
        opool = ctx.enter_context(tc.tile_pool(name="outp", bufs=3))
        rpool = ctx.enter_context(tc.tile_pool(name="rcp", bufs=3))
        ps_h = ctx.enter_context(tc.tile_pool(name="ps_h", bufs=3, space="PSUM"))
        ps_g = ctx.enter_context(tc.tile_pool(name="ps_g", bufs=1, space="PSUM"))
        ps_s = ctx.enter_context(tc.tile_pool(name="ps_s", bufs=1, space="PSUM"))
        ps_o = ctx.enter_context(tc.tile_pool(name="ps_o", bufs=3, space="PSUM"))

        W1s = const.tile([P, NCH, DC, CW], BF)
        W2s = const.tile([P, NEJ, O], BF)
        W2v = W2p.rearrange("(ec p) o -> p ec o", p=P)
        Wgs = const.tile([P, DC, 32], BF)
        cpks = const.tile([P, NEJ + 1], FP)
        b1s = cpks[:, 0:NEJ]
        bgs = cpks[0:E, NEJ : NEJ + 1]
        cms = const.tile([P, E], FPR)
        # b2 rows replicated at partition offset 32 so the second bias-seed
        # matmul can run in PE row group 1 (its operands stream rows 32-63)
        b2s = const.tile([32 + E, O], BF)
        ones8 = const.tile([E, 1], BF)

        def load_xts(blk, sync_dc=4):
            xts = xpool.tile([P, DC, T], BF, name=f"xts{blk}", tag="xts")
            xv = xT[:, blk, :, :]
            if sync_dc < DC:
                nc.sync.dma_start(xts[:, 0:sync_dc, :], xv[:, 0:sync_dc, :])
                nc.gpsimd.dma_start(xts[:, sync_dc:, :], xv[:, sync_dc:, :])
            else:
                nc.sync.dma_start(xts[:], xv[:])
            return xts

        def w1slice(c, r0, r1):
            return (W1s[:, c, r0:r1, :], W1p[:, c, r0:r1, :])

        # ---- prologue DMAs ----
        # Weights ride ONLY the SP and Pool queues. An engine stalls
        # in-stream when its DGE ring is full, and the ACT engine has the
        # gelu/exp stream right behind its dispatches -- W1 slices on the
        # ACT queue were observed to delay the first gelu by ~11us (PE
        # back-pressured via the PSUM ring). SP and Pool have no critical
        # compute, so they eat all the weight bytes; ACT gets one packed
        # const DMA up front and nothing else until the final block's
        # output writes. Queue byte streams follow the PE's consumption
        # order: xts0 -> Wgs -> W1 chunks -> xts1 -> b2/W2.
        # only the 8 real gate columns ride HBM; the 24 zero-pad columns
        # (which make the col-tiled gate write every PSUM row of its strip)
        # come from a memset, shaving 48KB off SP's critical path to the
        # first matmul
        nc.gpsimd.memset(Wgs[:], 0.0)
        nc.sync.dma_start(Wgs[:, :, 0:E], Wg[:])
        # Pool delivers W1 c0's tail BEFORE its xts0 share: the gate can't
        # start until all of xts0 lands anyway, but L1 needs c0 right after
        # the gate+bridge, and c0's sync share is the late one
        nc.gpsimd.dma_start(*w1slice(0, 5, 8))
        xts0 = xpool.tile([P, DC, T], BF, name="xts0", tag="xts")
        nc.sync.dma_start(xts0[:, 0:3, :], xT[:, 0, 0:3, :])
        nc.gpsimd.dma_start(xts0[:, 3:, :], xT[:, 0, 3:, :])
        xtss = {0: xts0}
        nc.scalar.dma_start(cpks[:], cpk[:])
        nc.scalar.dma_start(cms[:], cmat[:])
        nc.gpsimd.memset(ones8[:], 1.0)
        nc.sync.dma_start(*w1slice(0, 0, 5))

        def gate_logits(blk, xts):
            # gate logits (transposed): gt[e, t]. The gate only fills 8 of
            # 128 output partitions, so the 8 dc matmuls run as 4 CONCURRENT
            # column-group tiles (PE col tiling): strip j accumulates dc j
            # and dc j+4. The four partial sums are combined off-PE in
            # gate_exp_act via exp(a+b) = exp(a)*exp(b). PE cost per block
            # drops from 8 serial matmuls to ~2.
            gt = ps_g.tile([P, T], FP, name=f"gt{blk}", tag="gt")
            for dc in range(DC):
                j = 32 * (dc % 4)
                nc.tensor.matmul(
                    gt[j : j + 32, :],
                    Wgs[:, dc, :],
                    xts[:, dc, :],
                    start=(dc < 4),
                    stop=(dc >= 4),
                    tile_position=(0, j),
                )
            return gt

        def gate_collapse(blk, gt):
            # re-sum the four col-group partial strips exactly: ACT copies
            # the PSUM strips to SBUF, then a 0/1 [104,8] constant matmul
            # adds rows {e, 32+e, 64+e, 96+e} (fp32r moving, 1 cyc/row).
            # Placed after l2(k-1) in the PE stream so the copy is long done.
            gts = eqpool.tile([P, T], FPR, name=f"gts{blk}", tag="eq")
            nc.scalar.activation(gts[:], gt[:], AF.Copy)
            g2 = ps_g.tile([E, T], FP, name=f"g2_{blk}", tag="gt")
            nc.tensor.matmul(g2[:], cms[:], gts[:], start=True, stop=True)
            return g2

        def gate_exp_act(blk, g2):
            # exp rows at partitions 0-7; rows 32-39 get a copy via the DRAM
            # bounce so the second bias-seed matmul can use row group 1
            expv = epool.tile([32 + E, T], BF, name=f"exp{blk}", tag="exp")
            nc.scalar.activation(expv[0:E, :], g2[:], AF.Exp, bias=bgs[:, 0:1])
            return expv

        def gate_bcast(blk, expv):
            # broadcast exp rows across partitions for the hidden scaling
            # (partition-stride-0 DMA only legal from DRAM -> bounce there)
            expd = dpool.tile([E, T], BF, name=f"expd{blk}", tag="expd")
            nc.gpsimd.dma_start(expd[:], expv[0:E, :])
            nc.gpsimd.dma_start(expv[32 : 32 + E, :], expd[:])
            bc = bcpool.tile([P, E, T], BF, name=f"bc{blk}", tag="bc")
            nc.gpsimd.dma_start(bc[:], expd[:].partition_broadcast(P))
            return bc

        def gate_exp(blk, gt):
            expv = gate_exp_act(blk, gate_collapse(blk, gt))
            return expv, gate_bcast(blk, expv)

        def gate_denom(blk, expv):
            # softmax denominator, landed in token-partition layout via a
            # K=8 ones matmul; both 128-token halves into one PSUM tile
            s = ps_s.tile([P, TS], FP, name=f"s{blk}", tag="s")
            for ts in range(TS):
                nc.tensor.matmul(
                    s[:, ts : ts + 1],
                    expv[0:E, ts * P : (ts + 1) * P],
                    ones8[:],
                    start=True,
                    stop=True,
                )
            rcp = rpool.tile([P, TS], FP, name=f"rcp{blk}", tag="rcp")
            nc.vector.reciprocal(rcp[:], s[:])
            return rcp

        g0full = const.tile([P, NEJ, T], BF)

        def l1_stage(blk, xts, bc, defer=False):
            # defer=True (block 0 only): gelu outputs land in a dedicated
            # buffer and the exp-scaling DVE muls are emitted later -- the
            # block-0 exp broadcast rides the Pool queue AFTER the whole W1
            # (its 0.5MB would otherwise displace W1 bytes in the
            # HBM-bound window and the DVE wait would back-pressure the PE
            # through the gelu ring)
            sh = shpool.tile([P, NEJ, T], BF, name=f"sh{blk}", tag="sh")
            for ejc in range(NEJ):
                ht = ps_h.tile([P, T], FP, name=f"ht{blk}_{ejc}", tag="ht")
                for dc in range(DC):
                    nc.tensor.matmul(
                        ht[:],
                        W1s[:, ejc // 2, dc, (ejc % 2) * P : (ejc % 2 + 1) * P],
                        xts[:, dc, :],
                        start=(dc == 0),
                        stop=(dc == DC - 1),
                    )
                if defer:
                    nc.scalar.activation(
                        g0full[:, ejc, :], ht[:], AF.Gelu, bias=b1s[:, ejc : ejc + 1]
                    )
                else:
                    g = gpool.tile([P, T], BF, name=f"g{blk}_{ejc}", tag="g")
                    nc.scalar.activation(
                        g[:], ht[:], AF.Gelu, bias=b1s[:, ejc : ejc + 1]
                    )
                    nc.vector.tensor_tensor(
                        sh[:, ejc, :],
                        g[:],
                        bc[:, ejc // JC_PER_E, :],
                        mybir.AluOpType.mult,
                    )
            return sh

        def deferred_muls(sh, bc):
            for ejc in range(NEJ):
                nc.vector.tensor_tensor(
                    sh[:, ejc, :],
                    g0full[:, ejc, :],
                    bc[:, ejc // JC_PER_E, :],
                    mybir.AluOpType.mult,
                )

        # Outputs ride SP only (ACT's ring would block later gelus; Pool's
        # teardown DRAIN (~8.7us) runs after its last DMA). The final
        # block splits across SP+ACT -- no gelus follow, so ACT is free.
        def l2_stage(blk, sh, expv, rcp, last=False):
            t0 = blk * T
            for ts in range(TS):
                tsl = slice(ts * P, (ts + 1) * P)
                # the two K=8 bias seeds run concurrently in row groups 0/1
                ops = []
                for half in range(2):
                    o0 = half * OH
                    op = ps_o.tile(
                        [P, OH], FP, name=f"ops{blk}_{ts}_{half}", tag="ops"
                    )
                    r = 32 * half
                    nc.tensor.matmul(
                        op[:],
                        expv[r : r + E, tsl],
                        b2s[r : r + E, o0 : o0 + OH],
                        start=True,
                        stop=False,
                        tile_position=(r, 0),
                    )
                    ops.append(op)
                for half in range(2):
                    o0 = half * OH
                    for ejc in range(NEJ):
                        nc.tensor.matmul(
                            ops[half][:],
                            sh[:, ejc, tsl],
                            W2s[:, ejc, o0 : o0 + OH],
                            start=False,
                            stop=(ejc == NEJ - 1),
                        )
                for half in range(2):
                    o0 = half * OH
                    outsb = opool.tile(
                        [P, OH], FP, name=f"o{blk}_{ts}_{half}", tag="o"
                    )
                    # per-token 1/sum_e exp scale; DVE so ACT stays on gelu
                    nc.vector.tensor_scalar_mul(
                        outsb[:], ops[half][:], rcp[:, ts : ts + 1]
                    )
                    orows = out[t0 + ts * P : t0 + (ts + 1) * P, :]
                    if last and ts == TS - 1:
                        # final tiles: halve and alternate queues to shrink
                        # the post-compute drain
                        qh = OH // 2
                        eng = [nc.sync, nc.scalar] if half == 0 else [nc.scalar, nc.sync]
                        eng[0].dma_start(orows[:, o0 : o0 + qh], outsb[:, 0:qh])
                        eng[1].dma_start(orows[:, o0 + qh : o0 + OH], outsb[:, qh:OH])
                    else:
                        eng = nc.scalar if (last and half == 1) else nc.sync
                        eng.dma_start(orows[:, o0 : o0 + OH], outsb[:])

        with tc.high_priority():
            gt0 = gate_logits(0, xtss[0])
            expv0 = gate_exp_act(0, gate_collapse(0, gt0))
            # HAM warm-up bridge: the PE sits out a moment here waiting for
            # W1 chunk 0 (HBM-bound), which would keep the clock gate at
            # 1.2GHz deep into L1. Burn the wait on throwaway matmuls that
            # depend only on already-resident data so the activity monitor
            # sees a busy PE and releases full clock by the time real L1
            # work starts.
            dummy = ps_s.tile([E, T], FP, name="dummy", tag="s")
            for k in range(2):
                nc.tensor.matmul(
                    dummy[:],
                    Wgs[:, k % DC, 0:E],
                    xtss[0][:, (k + 1) % DC, :],
                    start=True,
                    stop=True,
                )
        for c in range(1, NCH):
            nc.sync.dma_start(*w1slice(c, 0, 5))
            nc.gpsimd.dma_start(*w1slice(c, 5, 8))
        bc0 = gate_bcast(0, expv0)
        states = {0: [expv0, None, bc0]}
        # x blocks 1-3 preloaded BEFORE W2: gate(k+1) runs right after
        # l1(k), and an x block queued behind W2's 4MB arrives ~10us late
        xtss[1] = load_xts(1)
        xtss[2] = load_xts(2)
        xtss[3] = load_xts(3)
        nc.sync.dma_start(b2s[0:E, :], b2[:])
        nc.sync.dma_start(b2s[32 : 32 + E, :], b2[:])

        def load_w2():
            # W2 in single-ejc-chunk slices, consumption-ordered, emitted at
            # the END of loop iteration 0 so block 1's exp broadcast
            # precedes it in the Pool queue (bc(1) behind W2's MBs starved
            # the block-1 DVE muls and back-pressured the PE)
            for sg in range(NEJ):
                eng = nc.gpsimd if sg % 3 == 2 else nc.sync
                eng.dma_start(W2s[:, sg : sg + 1, :], W2v[:, sg : sg + 1, :])

        sh_prev = None
        for blk in range(NB):
            sh = l1_stage(blk, xtss[blk], states[blk][2], defer=(blk == 0))
            if blk == 0:
                states[0][1] = gate_denom(0, states[0][0])
                deferred_muls(sh, states[0][2])
            if blk + 2 < NB and blk + 2 not in xtss:
                xtss[blk + 2] = load_xts(blk + 2)
            gt = gate_logits(blk + 1, xtss[blk + 1]) if blk + 1 < NB else None
            if blk >= 1:
                l2_stage(blk - 1, sh_prev, states[blk - 1][0], states[blk - 1][1])
            if blk + 1 < NB:
                expv, bc = gate_exp(blk + 1, gt)
                rcp = gate_denom(blk + 1, expv)
                states[blk + 1] = [expv, rcp, bc]
            if blk == 0:
                load_w2()
            sh_prev = sh
        l2_stage(NB - 1, sh_prev, states[NB - 1][0], states[NB - 1][1], last=True)
    return nc


_CACHE = {}


def kernel(**inputs):
    x = np.asarray(inputs["x"], dtype=np.float32)
    W1 = np.asarray(inputs["W1"], dtype=np.float32)
    b1 = np.asarray(inputs["b1"], dtype=np.float32)
    W2 = np.asarray(inputs["W2"], dtype=np.float32)
    b2 = np.asarray(inputs["b2"], dtype=np.float32)
    Wg = np.asarray(inputs["Wg"], dtype=np.float32)
    bg = np.asarray(inputs["bg"], dtype=np.float32)

    # W1 flat [D, EJ] -> partition-outer [P, NCH, DC, CW] so every DMA
    # slice is >=2KB contiguous per partition
    W1f = W1.transpose(1, 0, 2).reshape(D, EJ).astype(NPBF)
    W1p = np.ascontiguousarray(
        W1f.reshape(DC, P, NCH, CW).transpose(1, 2, 0, 3)
    )
    W2p = np.ascontiguousarray(W2.reshape(EJ, O)).astype(NPBF)
    cpk = np.zeros((P, NEJ + 1), dtype=np.float32)
    cpk[:, 0:NEJ] = b1.reshape(NEJ, P).T
    cpk[0:E, NEJ] = bg
    cmat = np.zeros((P, E), dtype=np.float32)
    for j in range(4):
        cmat[32 * j : 32 * j + E, :] = np.eye(E, dtype=np.float32)
    Wg16 = np.ascontiguousarray(
        Wg.astype(NPBF).reshape(DC, P, E).transpose(1, 0, 2)
    )
    b216 = b2.astype(NPBF)

    if "nc" not in _CACHE:
        _CACHE["nc"] = _build_nc()
    nc = _CACHE["nc"]

    in_maps = []
    for c in range(NCORES):
        xs = x[c * NTOK : (c + 1) * NTOK].astype(NPBF)
        # [NTOK, D] -> partition-outer [P, NB, DC, T]
        xq = np.ascontiguousarray(
            xs.reshape(NB, T, DC, P).transpose(3, 0, 2, 1)
        )
        in_maps.append(
            {
                "xT": xq,
                "W1p": W1p,
                "Wg": Wg16,
                "W2p": W2p,
                "cpk": cpk,
                "cmat": cmat,
                "b2": b216,
            }
        )

    res = run_bass_kernel_spmd(nc, in_maps, list(range(NCORES)))
    kernel.last = res
    return np.concatenate([res.results[c]["out"] for c in range(NCORES)], axis=0)
